# revision 12
# baseline (speedup 1.0000x reference)
"""Trainium2 Bass kernel v2 for DLUPack — "t-pack" design.

CARAFE as 13 matmuls per (h, c-half): contraction (t2, w64) packs two taps
per matmul; rhs mask "strips" are built 26-diagonals-at-a-time by one DVE
2x-mode op per h-row from a w-major, k-permuted mask tensor (muw). Output
lands c-major in PSUM and is DMA'd straight to DRAM.

Layouts (per core, h0 = qh*HB):
  XS0/XS1 [128, RX*W] bf16      c-major x halves (conv1 rhs)
  xTb     [64=w, RX*C] fp16     w-major x, free (r, c)
  X2[s]   [128=(t,w), HB*C] fp16  half t: xpad[c, h+ki_t, w+kj_t-2] at free (h, c)
  comp    [64, RX*66] bf16      conv1 out, col-pad 1, halo rows zeroed
  EO      [34, RM*66] fp16      rows 0-24 exp'd scores, 25 Zr, 26-33 off
  mTW[dxi][128, (r,s14,q,p)] fp16  transposed exp'd scores, k-permuted
                                   (s<13: half0 k=2s / half1 k=2s+1; s=13 unused)
  zW[dxi] [128, (r,q,p)] fp16   transposed 1/Z
  offW    [128, (r,8)] fp16     transposed offsets (both halves dup)
  wpr     [128, (dy,dx,h,q,p)] fp16  tri-weights x 1/Z
  muw     [128, (h,q,s13,p)] fp16    final per-tap mask values
  strip   [128, (q,w',s13,p)] fp16   = muw x IDrep2 (diag+vmask), per h
  psum    [128=c-half, (p,q,w')] f32 -> DMA out
"""

import numpy as np
import ml_dtypes

import concourse.bass as bass
import concourse.tile as tile
from concourse import bacc, mybir
from concourse.bass_utils import run_bass_kernel_spmd

F32 = mybir.dt.float32
BF16 = mybir.dt.bfloat16
FP16 = mybir.dt.float16
ALU = mybir.AluOpType
ACTF = mybir.ActivationFunctionType

N, C, H, W = 2, 256, 64, 64
S, K, CC = 2, 5, 64
HOUT, WOUT = H * S, W * S
QH = 4
HB = H // QH           # 16
RX = HB + 4            # 20
RM = HB + 2            # 18
NS = 13                # s<10: jj=s//5, taps ki*5+(2jj+t); s 10-12: kj=4 pairs (2i, 2i+1)
NCH = 40               # 25 ker + 1 Zr + 8 off (at 32:40, psum-aligned)
EOW = W + 2            # padded col pitch

_cache = {}


def _ki(k):
    return k // K


def _kj(k):
    return k % K


def _build(debug=False):
    nc = bacc.Bacc("TRN2", target_bir_lowering=False, debug=False,
                   num_devices=8)

    def din(name, shape, dt=F32):
        return nc.dram_tensor(name, shape, dt, kind="ExternalInput").ap()

    xs_h = din("xs_h", [C, RX * W], BF16)
    xtb_h = din("xtb_h", [W, RX * C], FP16)
    w1l = din("w1l", [C, CC], BF16)
    b1c = din("b1c", [CC, 1])
    w2l = din("w2l", [CC, 9 * NCH], BF16)
    b2c = din("b2c", [NCH, 1])
    idt34 = din("idt34", [NCH, NCH], FP16)
    idrep = din("idrep", [128, 2 * W * NS * 2], FP16)
    ylo_h = din("ylo_h", [128, HB * 4], FP16)   # (h,q,p) -(h0+h)
    yhi_h = din("yhi_h", [128, HB * 4], FP16)
    xlo_h = din("xlo_h", [128, 1])              # f32 scalars per partition
    xhi_h = din("xhi_h", [128, 1])
    rmEO = din("rmEO", [34, RM])                 # 1/0 per mask row (in-image)
    out_sl = nc.dram_tensor("out_sl", [C, 2 * HB, WOUT], F32,
                            kind="ExternalOutput").ap()
    if debug:
        dbg = {}
        for nm, sh, dt in [("d_comp", [CC, RX * EOW], BF16),
                           ("d_EO", [NCH, RM * EOW], FP16),
                           ("d_mTW1", [128, RM * 4 * 16], FP16),
                           ("d_zW1", [128, RM * 4], FP16),
                           ("d_offW", [128, RM * 8], FP16),
                           ("d_wpr", [128, 9 * HB * 4], FP16),
                           ("d_muw", [128, HB * 4 * NS], FP16),
                           ("d_strip0", [128, 2 * W * NS * 2], FP16),
                           ("d_X2_0", [128, HB * C], FP16)]:
            dbg[nm] = nc.dram_tensor(nm, sh, dt, kind="ExternalOutput").ap()

    with tile.TileContext(nc) as tc:
        with tc.tile_pool(name="per", bufs=1) as per, \
             tc.tile_pool(name="psA", bufs=2, space="PSUM") as psA, \
             tc.tile_pool(name="psT", bufs=2, space="PSUM") as psT, \
             tc.tile_pool(name="psC", bufs=4, space="PSUM") as psC:

            XS0 = per.tile([128, RX * W], BF16, tag="XS0")
            XS1 = per.tile([128, RX * W], BF16, tag="XS1")
            xTb = per.tile([W, RX * C], FP16, tag="xTb")
            w1a = per.tile([128, CC], BF16, tag="w1a")
            w1b = per.tile([128, CC], BF16, tag="w1b")
            b1t = per.tile([CC, 1], F32, tag="b1t")
            w2t = per.tile([CC, 9 * NCH], BF16, tag="w2t")
            b2t = per.tile([NCH, 1], F32, tag="b2t")
            idt = per.tile([NCH, NCH], FP16, tag="idt")
            idr = per.tile([128, 2 * W * NS * 2], FP16, tag="idr")
            ylo = per.tile([128, HB * 4], FP16, tag="ylo")
            yhi = per.tile([128, HB * 4], FP16, tag="yhi")
            xlo = per.tile([128, 1], F32, tag="xlo")
            xhi = per.tile([128, 1], F32, tag="xhi")
            rmt = per.tile([34, RM], F32, tag="rmt")

            comp = per.tile([CC, RX * EOW], BF16, tag="comp")
            EO = per.tile([NCH, RM * EOW], FP16, tag="EO")
            zrow = per.tile([1, RM * EOW], F32, tag="zrow")
            mTW = [per.tile([128, RM * 4 * 16], FP16, name=f"mTW{i}")
                   for i in range(3)]
            zW = [per.tile([128, RM * 4], FP16, name=f"zW{i}")
                  for i in range(3)]
            offW = per.tile([128, RM * 8], FP16, tag="offW")
            oyc = per.tile([128, HB * 4], FP16, tag="oyc")
            oxc = per.tile([128, HB * 4], FP16, tag="oxc")
            wy = per.tile([128, 3 * HB * 4], FP16, tag="wy")
            wx = per.tile([128, 3 * HB * 4], FP16, tag="wx")
            t192 = per.tile([128, 3 * HB * 4], FP16, tag="t192")
            wpr = per.tile([128, 9 * HB * 4], FP16, tag="wpr")
            muw = per.tile([128, HB * 4 * NS], FP16, tag="muw")
            mut = per.tile([128, HB * 4 * NS], FP16, tag="mut")
            X2 = [per.tile([128, RX * C], FP16, name=f"X2_{j}")
                  for j in range(3)]
            strips = [per.tile([128, 2 * W * NS * 2], FP16, name=f"strip{i}")
                      for i in range(3)]

            # ---------------- input DMAs ----------------
            nc.sync.dma_start(XS0[:], xs_h[0:128, :])
            nc.sync.dma_start(XS1[:], xs_h[128:256, :])
            nc.sync.dma_start(xTb[:], xtb_h[:])
            nc.sync.dma_start(w1a[:], w1l[0:128, :])
            nc.sync.dma_start(w1b[:], w1l[128:256, :])
            nc.sync.dma_start(b1t[:], b1c[:])
            nc.sync.dma_start(w2t[:], w2l[:])
            nc.sync.dma_start(b2t[:], b2c[:])
            nc.sync.dma_start(idt[:], idt34[:])
            nc.sync.dma_start(idr[:], idrep[:])
            nc.sync.dma_start(ylo[:], ylo_h[:])
            nc.sync.dma_start(yhi[:], yhi_h[:])
            nc.sync.dma_start(xlo[:], xlo_h[:])
            nc.sync.dma_start(xhi[:], xhi_h[:])
            nc.sync.dma_start(rmt[:], rmEO[:])
            rmc = per.tile([CC, RX], F32, tag="rmc")
            nc.sync.dma_start(rmc[:], din("rmc_h", [CC, RX]))

            # ------- X2 rotation-pair tiles (SB->SB): halves (kj=2jj+t) -------
            dmaq = [nc.sync, nc.scalar, nc.gpsimd]
            qi = 0
            nc.gpsimd.memset(X2[2][64:128, (RX - 1) * C:], 0.0)
            for j in range(3):
                for t in range(2):
                    kj = 2 * j + t if j < 2 else 4
                    rot = (kj - 2) % W
                    n0 = W - rot
                    for (d0, s0, cnt) in ((0, rot, n0), (n0, 0, rot)):
                        if cnt == 0:
                            continue
                        if j == 2 and t == 1:
                            # kj=4 singles pair via +1-row stagger in half1
                            dmaq[qi % 3].dma_start(
                                X2[j][t * 64 + d0:t * 64 + d0 + cnt,
                                      0:(RX - 1) * C],
                                xTb[s0:s0 + cnt, C:])
                        else:
                            dmaq[qi % 3].dma_start(
                                X2[j][t * 64 + d0:t * 64 + d0 + cnt, :],
                                xTb[s0:s0 + cnt, :])
                        qi += 1

            # ---------------- conv1 ----------------
            nc.gpsimd.memset(comp[:], 0.0)
            cpv = comp[:].rearrange("p (r w) -> p r w", r=RX)
            xs0v = XS0[:].rearrange("p (r w) -> p r w", r=RX)
            xs1v = XS1[:].rearrange("p (r w) -> p r w", r=RX)
            for i in range(4):
                r0 = i * 5
                p1 = psA.tile([CC, 5 * W], F32, tag="cv", name=f"p1_{i}")
                nc.tensor.matmul(p1[:], w1a[:], xs0v[:, r0:r0 + 5, :],
                                 start=True, stop=False)
                nc.tensor.matmul(p1[:], w1b[:], xs1v[:, r0:r0 + 5, :],
                                 start=False, stop=True)
                nc.vector.tensor_scalar(
                    cpv[:, r0:r0 + 5, 1:65],
                    p1[:].rearrange("p (r w) -> p r w", r=5),
                    b1t[:, 0:1], None, op0=ALU.add)
                nc.vector.tensor_tensor(
                    cpv[:, r0:r0 + 5, 1:65], cpv[:, r0:r0 + 5, 1:65],
                    rmc[:, r0:r0 + 5].unsqueeze(2).broadcast_to([CC, 5, W]),
                    op=ALU.mult)
            # zero halo rows outside image: host passes which rows via memset
            # list encoded in rmEO? simpler: zero rows where rm=0 using mult
            # over the (r) dim is free-dim -> do per-row memsets host-known.
            # We instead zero on host side by rmask? cannot (device data).
            # Do: multiply comp by row mask broadcast along free r dim:
            # comp cols (r, w): in1 = rmt-derived not available per r on
            # partitions... use memset per OOB row (max 2 rows, known at
            # compile? row validity depends on qh -> runtime per-core!).
            # Trick: host sends xs already zeroed in halo; conv1 of zeros is
            # zero + BIAS added -> must re-zero. Use rmc mask tile instead:


            # ---------------- conv2 ----------------
            w2v = w2t[:].rearrange("p (t o) -> p t o", t=9)
            eov = EO[:].rearrange("p (r w) -> p r w", r=RM)
            for i in range(3):
                r0 = i * 6
                p2 = psA.tile([NCH, 6 * W], F32, tag="cv", name=f"p2_{i}")
                for t in range(9):
                    dy, dx = t // 3, t % 3
                    nc.tensor.matmul(
                        p2[:].rearrange("p (r w) -> p r w", r=6),
                        w2v[:, t, :],
                        cpv[:, r0 + dy:r0 + dy + 6, dx:dx + W],
                        start=(t == 0), stop=(t == 8))
                # scores rows 0..24: exp(x + b); off rows 26..33: x + b
                nc.scalar.activation(
                    eov[0:25, r0:r0 + 6, 1:65],
                    p2[0:25, :].rearrange("p (r w) -> p r w", r=6),
                    ACTF.Exp, bias=b2t[0:25, 0:1])
                nc.vector.tensor_scalar(
                    eov[32:40, r0:r0 + 6, 1:65],
                    p2[32:40, :].rearrange("p (r w) -> p r w", r=6),
                    b2t[32:40, 0:1], None, op0=ALU.add)
            # zero pad cols of score rows (cols 0 and 65 of each r)
            nc.gpsimd.memset(eov[0:25, :, 0:1], 0.0)
            nc.gpsimd.memset(eov[0:25, :, 65:66], 0.0)
            # mask out-of-image rows: mult scores by rmt row-mask (free dim)
            nc.vector.tensor_tensor(
                eov[0:25, :, 1:65], eov[0:25, :, 1:65],
                rmt[0:25, :].unsqueeze(2).broadcast_to([25, RM, W]),
                op=ALU.mult)

            # Z = sum_k E -> Zr = rm / max(Z, 1)
            ones25 = per.tile([25, 1], FP16, tag="ones25")
            nc.gpsimd.memset(ones25[:], 1.0)
            zrv = zrow[:].rearrange("p (r w) -> p r w", r=RM)
            for g in range(3):
                c0 = g * 6 * EOW
                pz = psA.tile([1, 6 * EOW], F32, tag="cv", name=f"pz_{g}")
                nc.tensor.matmul(pz[:], ones25[:],
                                 EO[0:25, c0:c0 + 6 * EOW],
                                 start=True, stop=True)
                nc.vector.tensor_scalar_max(zrow[:, c0:c0 + 6 * EOW],
                                            pz[:], 1.0)
                nc.vector.reciprocal(zrow[:, c0:c0 + 6 * EOW],
                                     zrow[:, c0:c0 + 6 * EOW])
                nc.vector.tensor_tensor(
                    zrv[:, g * 6:g * 6 + 6, 1:65],
                    zrv[:, g * 6:g * 6 + 6, 1:65],
                    rmt[0:1, g * 6:g * 6 + 6].unsqueeze(2)
                    .broadcast_to([1, 6, W]),
                    op=ALU.mult)
                nc.gpsimd.dma_start(EO[25:26, c0:c0 + 6 * EOW],
                                    zrow[:, c0:c0 + 6 * EOW])

            # ---------------- transposes -> w-major ----------------
            # mTW[dxi][(t,w), (r, s14, q, p)], zW, offW
            ofv = offW[:].rearrange("p (r c) -> p r c", r=RM)
            for g in range(3):          # r batches of 6
                r0 = g * 6
                for dxi in range(3):
                    ptd = psT.tile([W, 6 * NCH], FP16, tag="ptd",
                                   name=f"ptd{dxi}_{g}", bufs=2)
                    for j in range(6):
                        r = r0 + j
                        nc.tensor.transpose(
                            ptd[:, j * NCH:(j + 1) * NCH],
                            EO[:, r * EOW + dxi:r * EOW + dxi + W],
                            idt[:])
                    pv = ptd[:].rearrange("w (j c) -> w j c", j=6)
                    mv4 = mTW[dxi][:].rearrange("p (r f s) -> p r f s",
                                                r=RM, f=4)
                    zv4 = zW[dxi][:].rearrange("p (r f) -> p r f", r=RM)
                    for t in range(2):
                        for jj in range(3):
                            if jj < 2:
                                c0 = 2 * jj + t
                                ssl = mv4[t * 64:(t + 1) * 64, r0:r0 + 6, :,
                                          jj * 5:jj * 5 + 5]
                                src = pv[:, :, c0:c0 + 21:5]  # 5 ki str 5
                            elif t == 0:
                                ssl = mv4[0:64, r0:r0 + 6, :, 10:13]
                                src = pv[:, :, 4:25:10]       # ki 0,2,4
                            else:
                                ssl = mv4[64:128, r0:r0 + 6, :, 10:12]
                                src = pv[:, :, 9:20:10]       # ki 1,3
                            nsl = ssl.shape[3]
                            if (t + jj) % 2 == 0:
                                nc.scalar.activation(
                                    ssl,
                                    src.unsqueeze(2)
                                    .broadcast_to([W, 6, 4, nsl]),
                                    ACTF.Copy)
                            else:
                                nc.vector.tensor_copy(
                                    ssl,
                                    src.unsqueeze(2)
                                    .broadcast_to([W, 6, 4, nsl]))
                        nc.scalar.activation(
                            zv4[t * 64:(t + 1) * 64, r0:r0 + 6, :],
                            pv[:, :, 25:26].broadcast_to([W, 6, 4]),
                            ACTF.Copy)
                        if dxi == 1:
                            nc.scalar.activation(
                                ofv[t * 64:(t + 1) * 64, r0:r0 + 6, :],
                                pv[:, :, 32:40], ACTF.Copy)
            for dxi in range(3):
                mv4 = mTW[dxi][:].rearrange("p (r f s) -> p r f s",
                                            r=RM, f=4)
                nc.gpsimd.memset(mv4[64:128, :, :, 12:13], 0.0)


            # ---------------- WGT (w-major) ----------------
            oyv = oyc[:].rearrange("p (h q e) -> p h q e", h=HB, q=2)
            oxv = oxc[:].rearrange("p (h q e) -> p h q e", h=HB, q=2)
            # oy chan = q*4+2+p, ox chan = q*4+0+p at mask row h+1
            oy_src = ofv[:, 1:1 + HB, :].rearrange("p h (q c) -> p h q c", q=2)
            nc.vector.tensor_tensor(
                oyv[:], oy_src[:, :, :, 2:4],
                ylo[:].rearrange("p (h q e) -> p h q e", h=HB, q=2),
                op=ALU.max)
            nc.vector.tensor_tensor(
                oyv[:], oyv[:],
                yhi[:].rearrange("p (h q e) -> p h q e", h=HB, q=2),
                op=ALU.min)
            nc.vector.tensor_scalar(oxv[:], oy_src[:, :, :, 0:2],
                                    xlo[:, 0:1], xhi[:, 0:1],
                                    op0=ALU.max, op1=ALU.min)
            # wy[dy, h, q, p] = relu(1 - |oyc - (dy-1)|)
            for (wt, oc) in ((wy, oyc), (wx, oxc)):
                wv = wt[:].rearrange("p (d f) -> p d f", d=3)
                tv = t192[:].rearrange("p (d f) -> p d f", d=3)
                for d in range(3):
                    nc.vector.tensor_scalar(tv[:, d, :], oc[:],
                                            float(1 - d), None, op0=ALU.add)
                nc.vector.tensor_scalar(wv[:], tv[:], -1.0, None,
                                        op0=ALU.mult)
                nc.vector.tensor_tensor(wv[:], wv[:], tv[:], op=ALU.max)
                nc.vector.tensor_scalar(wv[:], wv[:], -1.0, 1.0,
                                        op0=ALU.mult, op1=ALU.add)
                nc.vector.tensor_scalar(wv[:], wv[:], 0.0, None, op0=ALU.max)
            # wpr[dy,dx,(h,q,p)] = wy*wx*zr[h+dy, dx]
            nc.vector.tensor_tensor(
                wpr[:].rearrange("p (a b f) -> p a b f", a=3, b=3),
                wy[:].rearrange("p (a f) -> p a f", a=3).unsqueeze(2)
                .broadcast_to([128, 3, 3, HB * 4]),
                wx[:].rearrange("p (b f) -> p b f", b=3).unsqueeze(1)
                .broadcast_to([128, 3, 3, HB * 4]),
                op=ALU.mult)
            wprv4 = wpr[:].rearrange("p (a b h f) -> p a b h f", a=3, b=3,
                                     h=HB)
            for dy in range(3):
                for dx in range(3):
                    zs = zW[dx][:].rearrange("p (r f) -> p r f", r=RM)
                    nc.vector.tensor_tensor(
                        wprv4[:, dy, dx], wprv4[:, dy, dx],
                        zs[:, dy:dy + HB, :], op=ALU.mult)

            # ---------------- MU (muw) ----------------
            # muw layout (h, f4=(q,p), s13); mTW (r, f4, s14)
            muv = muw[:].rearrange("p (h f s) -> p h f s", h=HB, f=4)
            mutv = mut[:].rearrange("p (h f s) -> p h f s", h=HB, f=4)
            for hc in range(4):
                hl = hc * 4
                first = True
                for dy in range(3):
                    for dx in range(3):
                        msrc = mTW[dx][:].rearrange(
                            "p (r f s) -> p r f s", r=RM, f=4)[
                            :, dy + hl:dy + hl + 4, :, 0:NS]
                        wsl = wprv4[:, dy, dx, hl:hl + 4, :].unsqueeze(3) \
                            .broadcast_to([128, 4, 4, NS])
                        dst = muv if first else mutv
                        nc.vector.tensor_tensor(dst[:, hl:hl + 4], wsl,
                                                msrc, op=ALU.mult)
                        if not first:
                            nc.vector.tensor_tensor(
                                muv[:, hl:hl + 4], muv[:, hl:hl + 4],
                                mutv[:, hl:hl + 4], op=ALU.add)
                        first = False

            # ---------------- CARAFE ----------------
            idrv = idr[:].rearrange("p (q w g) -> p q w g", q=2, w=W)
            muq = muw[:].rearrange("p (h q g) -> p h q g", h=HB, q=2)
            for h in range(HB):
                st = strips[h % 3]
                stv = st[:].rearrange("p (q w g) -> p q w g", q=2, w=W)
                nc.vector.tensor_tensor(
                    stv[:],
                    muq[:, h, :, :].unsqueeze(2)
                    .broadcast_to([128, 2, W, 2 * NS]),
                    idrv[:], op=ALU.mult)
                for ch in range(2):
                    pc = psC.tile([128, 2 * 128], F32, tag="pc",
                                  name=f"pc{ch}_{h}", bufs=4)
                    x2v = None
                    stv2 = st[:].rearrange("p (q w e s) -> p q w e s",
                                           q=2, w=W, e=2)
                    for s in range(NS):
                        jj = s // 5 if s < 10 else 2
                        ki = s % 5 if s < 10 else 2 * (s - 10)
                        lhsT = X2[jj][:, (h + ki) * C + ch * 128:
                                      (h + ki) * C + ch * 128 + 128]
                        rhs = stv2[:, :, :, :, s].transpose([0, 3, 1, 2])
                        nc.tensor.matmul(pc[:], lhsT, rhs,
                                         start=(s == 0), stop=(s == NS - 1))
                    # psum free = (p, q, w) -> out[c, 2h+p, (w,q)]
                    oc = per.tile([128, 256], F32, tag=f"oc{ch}",
                                  name=f"oc{ch}_{h}", bufs=3)
                    nc.scalar.activation(
                        oc[:].rearrange("c (e w q) -> c e w q", e=2, q=2),
                        pc[:].rearrange("c (e q w) -> c e q w", e=2, q=2)
                        .transpose([0, 1, 3, 2]),
                        ACTF.Copy)
                    ov = out_sl.rearrange("c r w -> c (r w)")
                    nc.sync.dma_start(
                        ov[ch * 128:(ch + 1) * 128,
                           2 * h * WOUT:(2 * h + 2) * WOUT],
                        oc[:])

            if debug:
                nc.sync.dma_start(dbg["d_comp"], comp[:])
                nc.sync.dma_start(dbg["d_EO"], EO[:])
                nc.sync.dma_start(dbg["d_mTW1"], mTW[1][:])
                nc.sync.dma_start(dbg["d_zW1"], zW[1][:])
                nc.sync.dma_start(dbg["d_offW"], offW[:])
                nc.sync.dma_start(dbg["d_wpr"], wpr[:])
                nc.sync.dma_start(dbg["d_muw"], muw[:])
                nc.sync.dma_start(dbg["d_strip0"], strips[0][:])
                nc.sync.dma_start(dbg["d_X2_0"], X2[0][:])

    nc.compile()
    return nc


def _consts(n, qh):
    h0 = qh * HB
    wv = np.arange(W)
    # ylo/yhi per (h,q,p): -(h0+h), 63-(h0+h)
    hvals = (h0 + np.arange(HB, dtype=np.float32))
    ylo = np.broadcast_to(np.repeat(-hvals, 4)[None, :], (128, HB * 4))
    yhi = np.broadcast_to(np.repeat(63.0 - hvals, 4)[None, :], (128, HB * 4))
    wq = np.tile(wv, 2).astype(np.float32)
    xlo = (-wq)[:, None].copy()
    xhi = (63.0 - wq)[:, None].copy()
    # IDrep2 [(t,w), (q, w', s, p)] = (w' == w%64) & valid(w', kj(t,s))
    idrep = np.zeros((128, 2, W, 2, NS), np.float32)
    for t in range(2):
        for s in range(NS):
            if s < 10:
                kj = 2 * (s // 5) + t
            else:
                if t == 1 and s == 12:
                    continue
                kj = 4
            for w in range(W):
                if 0 <= w + kj - 2 < W:
                    idrep[t * 64 + w, :, w, :, s] = 1.0
    # rmEO: mask row r (global h0-1+r) in image
    rm = np.array([[1.0 if 0 <= h0 - 1 + r < H else 0.0]
                   for r in range(RM)], np.float32).reshape(1, RM)
    rmc = np.zeros((CC, RX), np.float32)
    for r in range(RX):
        rmc[:, r] = 1.0 if 0 <= h0 - 2 + r < H else 0.0
    rm34 = np.broadcast_to(rm, (34, RM)).copy()
    return dict(ylo_h=ylo.astype(np.float16).copy(),
                yhi_h=yhi.astype(np.float16).copy(),
                xlo_h=xlo, xhi_h=xhi,
                idrep=idrep.reshape(128, -1).astype(np.float16),
                rmEO=rm34, rmc_h=rmc)


def _prep_in_maps(x, w_comp, b_comp, w_off, b_off, w_ker, b_ker):
    x = np.asarray(x, np.float32)
    w1l = np.asarray(w_comp, np.float32).reshape(CC, C).T.astype(
        ml_dtypes.bfloat16)
    perm = [xy * 4 + p * 2 + q for q in range(2) for xy in range(2)
            for p in range(2)]
    w2 = np.zeros((NCH, CC, 3, 3), np.float32)
    b2 = np.zeros((NCH,), np.float32)
    w2[0:25] = np.asarray(w_ker, np.float32)
    b2[0:25] = np.asarray(b_ker, np.float32)
    w2[32:40] = np.asarray(w_off, np.float32)[perm]
    b2[32:40] = np.asarray(b_off, np.float32)[perm]
    w2l = np.ascontiguousarray(
        w2.transpose(1, 2, 3, 0).reshape(CC, 9 * NCH)).astype(
        ml_dtypes.bfloat16)
    idt34 = np.eye(NCH, dtype=np.float16)

    in_maps = []
    for core in range(8):
        n, qh = core // QH, core % QH
        h0 = qh * HB
        lo, hi = h0 - 2, h0 + HB + 2
        slo, shi = max(lo, 0), min(hi, H)
        xs = np.zeros((C, RX, W), np.float32)
        xs[:, slo - lo:shi - lo] = x[n, :, slo:shi]
        xtb = np.ascontiguousarray(xs.transpose(2, 1, 0)).reshape(
            W, RX * C).astype(np.float16)
        im = dict(xs_h=xs.reshape(C, RX * W).astype(ml_dtypes.bfloat16),
                  xtb_h=xtb, w1l=w1l,
                  b1c=np.asarray(b_comp, np.float32)[:, None].copy(),
                  w2l=w2l, b2c=b2[:, None].copy(), idt34=idt34,
                  **_consts(n, qh))
        in_maps.append(im)
    return in_maps


def kernel(x, w_comp, b_comp, w_off, b_off, w_ker, b_ker):
    if "nc" not in _cache:
        _cache["nc"] = _build(debug=_cache.get("debug", False))
    nc = _cache["nc"]
    in_maps = _prep_in_maps(x, w_comp, b_comp, w_off, b_off, w_ker, b_ker)
    res = run_bass_kernel_spmd(nc, in_maps, core_ids=list(range(8)))
    _cache["last_res"] = res
    out = np.zeros((N, C, HOUT, WOUT), np.float32)
    for core in range(8):
        n, qh = core // QH, core % QH
        out[n, :, 2 * qh * HB:2 * (qh + 1) * HB] = res.results[core]["out_sl"]
    return out


# revision 14
# speedup vs baseline: 1.0006x; 1.0006x over previous
"""Trainium2 Bass kernel v2 for DLUPack — "t-pack" design.

CARAFE as 13 matmuls per (h, c-half): contraction (t2, w64) packs two taps
per matmul; rhs mask "strips" are built 26-diagonals-at-a-time by one DVE
2x-mode op per h-row from a w-major, k-permuted mask tensor (muw). Output
lands c-major in PSUM and is DMA'd straight to DRAM.

Layouts (per core, h0 = qh*HB):
  XS0/XS1 [128, RX*W] bf16      c-major x halves (conv1 rhs)
  xTb     [64=w, RX*C] fp16     w-major x, free (r, c)
  X2[s]   [128=(t,w), HB*C] fp16  half t: xpad[c, h+ki_t, w+kj_t-2] at free (h, c)
  comp    [64, RX*66] bf16      conv1 out, col-pad 1, halo rows zeroed
  EO      [34, RM*66] fp16      rows 0-24 exp'd scores, 25 Zr, 26-33 off
  mTW[dxi][128, (r,s14,q,p)] fp16  transposed exp'd scores, k-permuted
                                   (s<13: half0 k=2s / half1 k=2s+1; s=13 unused)
  zW[dxi] [128, (r,q,p)] fp16   transposed 1/Z
  offW    [128, (r,8)] fp16     transposed offsets (both halves dup)
  wpr     [128, (dy,dx,h,q,p)] fp16  tri-weights x 1/Z
  muw     [128, (h,q,s13,p)] fp16    final per-tap mask values
  strip   [128, (q,w',s13,p)] fp16   = muw x IDrep2 (diag+vmask), per h
  psum    [128=c-half, (p,q,w')] f32 -> DMA out
"""

import numpy as np
import ml_dtypes

import concourse.bass as bass
import concourse.tile as tile
from concourse import bacc, mybir
from concourse.bass_utils import run_bass_kernel_spmd

F32 = mybir.dt.float32
BF16 = mybir.dt.bfloat16
FP16 = mybir.dt.float16
ALU = mybir.AluOpType
ACTF = mybir.ActivationFunctionType

N, C, H, W = 2, 256, 64, 64
S, K, CC = 2, 5, 64
HOUT, WOUT = H * S, W * S
QH = 4
HB = H // QH           # 16
RX = HB + 4            # 20
RM = HB + 2            # 18
NS = 13                # s<10: jj=s//5, taps ki*5+(2jj+t); s 10-12: kj=4 pairs (2i, 2i+1)
NCH = 40               # 25 ker + 1 Zr + 8 off (at 32:40, psum-aligned)
EOW = W + 2            # padded col pitch

_cache = {}


def _ki(k):
    return k // K


def _kj(k):
    return k % K


def _build(debug=False):
    nc = bacc.Bacc("TRN2", target_bir_lowering=False, debug=False,
                   num_devices=8)

    def din(name, shape, dt=F32):
        return nc.dram_tensor(name, shape, dt, kind="ExternalInput").ap()

    xs_h = din("xs_h", [C, RX * W], BF16)
    xtb_h = din("xtb_h", [W, RX * C], FP16)
    w1l = din("w1l", [C, CC], BF16)
    b1c = din("b1c", [CC, 1])
    w2l = din("w2l", [CC, 9 * NCH], BF16)
    b2c = din("b2c", [NCH, 1])
    idt34 = din("idt34", [NCH, NCH], FP16)
    idrep = din("idrep", [128, 2 * W * NS * 2], FP16)
    ylo_h = din("ylo_h", [128, HB * 4], FP16)   # (h,q,p) -(h0+h)
    yhi_h = din("yhi_h", [128, HB * 4], FP16)
    xlo_h = din("xlo_h", [128, 1])              # f32 scalars per partition
    xhi_h = din("xhi_h", [128, 1])
    rmEO = din("rmEO", [34, RM])                 # 1/0 per mask row (in-image)
    out_sl = nc.dram_tensor("out_sl", [C, 2 * HB, WOUT], F32,
                            kind="ExternalOutput").ap()
    if debug:
        dbg = {}
        for nm, sh, dt in [("d_comp", [CC, RX * EOW], BF16),
                           ("d_EO", [NCH, RM * EOW], FP16),
                           ("d_mTW1", [128, RM * 4 * 16], FP16),
                           ("d_zW1", [128, RM * 4], FP16),
                           ("d_offW", [128, RM * 8], FP16),
                           ("d_wpr", [128, 9 * HB * 4], FP16),
                           ("d_muw", [128, HB * 4 * NS], FP16),
                           ("d_strip0", [128, 2 * W * NS * 2], FP16),
                           ("d_X2_0", [128, HB * C], FP16)]:
            dbg[nm] = nc.dram_tensor(nm, sh, dt, kind="ExternalOutput").ap()

    with tile.TileContext(nc) as tc:
        with tc.tile_pool(name="per", bufs=1) as per, \
             tc.tile_pool(name="psA", bufs=2, space="PSUM") as psA, \
             tc.tile_pool(name="psT", bufs=2, space="PSUM") as psT, \
             tc.tile_pool(name="psC", bufs=4, space="PSUM") as psC:

            XS0 = per.tile([128, RX * W], BF16, tag="XS0")
            XS1 = per.tile([128, RX * W], BF16, tag="XS1")
            xTb = per.tile([W, RX * C], FP16, tag="xTb")
            w1a = per.tile([128, CC], BF16, tag="w1a")
            w1b = per.tile([128, CC], BF16, tag="w1b")
            b1t = per.tile([CC, 1], F32, tag="b1t")
            w2t = per.tile([CC, 9 * NCH], BF16, tag="w2t")
            b2t = per.tile([NCH, 1], F32, tag="b2t")
            idt = per.tile([NCH, NCH], FP16, tag="idt")
            idr = per.tile([128, 2 * W * NS * 2], FP16, tag="idr")
            ylo = per.tile([128, HB * 4], FP16, tag="ylo")
            yhi = per.tile([128, HB * 4], FP16, tag="yhi")
            xlo = per.tile([128, 1], F32, tag="xlo")
            xhi = per.tile([128, 1], F32, tag="xhi")
            rmt = per.tile([34, RM], F32, tag="rmt")

            comp = per.tile([CC, RX * EOW], BF16, tag="comp")
            EO = per.tile([NCH, RM * EOW], FP16, tag="EO")
            zrow = per.tile([1, RM * EOW], F32, tag="zrow")
            mTW = [per.tile([128, RM * 4 * 16], FP16, name=f"mTW{i}")
                   for i in range(3)]
            zW = [per.tile([128, RM * 4], FP16, name=f"zW{i}")
                  for i in range(3)]
            offW = per.tile([128, RM * 8], FP16, tag="offW")
            oyc = per.tile([128, HB * 4], FP16, tag="oyc")
            oxc = per.tile([128, HB * 4], FP16, tag="oxc")
            wy = per.tile([128, 3 * HB * 4], FP16, tag="wy")
            wx = per.tile([128, 3 * HB * 4], FP16, tag="wx")
            t192 = per.tile([128, 3 * HB * 4], FP16, tag="t192")
            wpr = per.tile([128, 9 * HB * 4], FP16, tag="wpr")
            muw = per.tile([128, HB * 4 * NS], FP16, tag="muw")
            mut = per.tile([128, HB * 4 * NS], FP16, tag="mut")
            X2 = [per.tile([128, RX * C], FP16, name=f"X2_{j}")
                  for j in range(3)]
            strips = [per.tile([128, 2 * W * NS * 2], FP16, name=f"strip{i}")
                      for i in range(3)]

            # ---------------- input DMAs ----------------
            nc.sync.dma_start(XS0[:], xs_h[0:128, :])
            nc.sync.dma_start(XS1[:], xs_h[128:256, :])
            nc.sync.dma_start(xTb[:], xtb_h[:])
            nc.sync.dma_start(w1a[:], w1l[0:128, :])
            nc.sync.dma_start(w1b[:], w1l[128:256, :])
            nc.sync.dma_start(b1t[:], b1c[:])
            nc.sync.dma_start(w2t[:], w2l[:])
            nc.sync.dma_start(b2t[:], b2c[:])
            nc.sync.dma_start(idt[:], idt34[:])
            nc.sync.dma_start(idr[:], idrep[:])
            nc.sync.dma_start(ylo[:], ylo_h[:])
            nc.sync.dma_start(yhi[:], yhi_h[:])
            nc.sync.dma_start(xlo[:], xlo_h[:])
            nc.sync.dma_start(xhi[:], xhi_h[:])
            nc.sync.dma_start(rmt[:], rmEO[:])
            rmc = per.tile([CC, RX], F32, tag="rmc")
            nc.sync.dma_start(rmc[:], din("rmc_h", [CC, RX]))

            # ------- X2 rotation-pair tiles (SB->SB): halves (kj=2jj+t) -------
            dmaq = [nc.sync, nc.scalar, nc.gpsimd]
            qi = 0
            nc.gpsimd.memset(X2[2][64:128, (RX - 1) * C:], 0.0)
            for j in range(3):
                for t in range(2):
                    kj = 2 * j + t if j < 2 else 4
                    rot = (kj - 2) % W
                    n0 = W - rot
                    for (d0, s0, cnt) in ((0, rot, n0), (n0, 0, rot)):
                        if cnt == 0:
                            continue
                        if j == 2 and t == 1:
                            # kj=4 singles pair via +1-row stagger in half1
                            dmaq[qi % 3].dma_start(
                                X2[j][t * 64 + d0:t * 64 + d0 + cnt,
                                      0:(RX - 1) * C],
                                xTb[s0:s0 + cnt, C:])
                        else:
                            dmaq[qi % 3].dma_start(
                                X2[j][t * 64 + d0:t * 64 + d0 + cnt, :],
                                xTb[s0:s0 + cnt, :])
                        qi += 1

            # ---------------- conv1 ----------------
            nc.gpsimd.memset(comp[:], 0.0)
            cpv = comp[:].rearrange("p (r w) -> p r w", r=RX)
            xs0v = XS0[:].rearrange("p (r w) -> p r w", r=RX)
            xs1v = XS1[:].rearrange("p (r w) -> p r w", r=RX)
            for i in range(4):
                r0 = i * 5
                p1 = psA.tile([CC, 5 * W], F32, tag="cv", name=f"p1_{i}")
                nc.tensor.matmul(p1[:], w1a[:], xs0v[:, r0:r0 + 5, :],
                                 start=True, stop=False)
                nc.tensor.matmul(p1[:], w1b[:], xs1v[:, r0:r0 + 5, :],
                                 start=False, stop=True)
                nc.vector.tensor_scalar(
                    cpv[:, r0:r0 + 5, 1:65],
                    p1[:].rearrange("p (r w) -> p r w", r=5),
                    b1t[:, 0:1], None, op0=ALU.add)
                nc.vector.tensor_tensor(
                    cpv[:, r0:r0 + 5, 1:65], cpv[:, r0:r0 + 5, 1:65],
                    rmc[:, r0:r0 + 5].unsqueeze(2).broadcast_to([CC, 5, W]),
                    op=ALU.mult)
            # zero halo rows outside image: host passes which rows via memset
            # list encoded in rmEO? simpler: zero rows where rm=0 using mult
            # over the (r) dim is free-dim -> do per-row memsets host-known.
            # We instead zero on host side by rmask? cannot (device data).
            # Do: multiply comp by row mask broadcast along free r dim:
            # comp cols (r, w): in1 = rmt-derived not available per r on
            # partitions... use memset per OOB row (max 2 rows, known at
            # compile? row validity depends on qh -> runtime per-core!).
            # Trick: host sends xs already zeroed in halo; conv1 of zeros is
            # zero + BIAS added -> must re-zero. Use rmc mask tile instead:


            # ---------------- conv2 ----------------
            w2v = w2t[:].rearrange("p (t o) -> p t o", t=9)
            eov = EO[:].rearrange("p (r w) -> p r w", r=RM)
            for i in range(3):
                r0 = i * 6
                p2 = psA.tile([NCH, 6 * W], F32, tag="cv", name=f"p2_{i}")
                for t in range(9):
                    dy, dx = t // 3, t % 3
                    nc.tensor.matmul(
                        p2[:].rearrange("p (r w) -> p r w", r=6),
                        w2v[:, t, :],
                        cpv[:, r0 + dy:r0 + dy + 6, dx:dx + W],
                        start=(t == 0), stop=(t == 8))
                # scores rows 0..24: exp(x + b); off rows 26..33: x + b
                nc.scalar.activation(
                    eov[0:25, r0:r0 + 6, 1:65],
                    p2[0:25, :].rearrange("p (r w) -> p r w", r=6),
                    ACTF.Exp, bias=b2t[0:25, 0:1])
                nc.vector.tensor_scalar(
                    eov[32:40, r0:r0 + 6, 1:65],
                    p2[32:40, :].rearrange("p (r w) -> p r w", r=6),
                    b2t[32:40, 0:1], None, op0=ALU.add)
            # zero pad cols of score rows (cols 0 and 65 of each r)
            nc.gpsimd.memset(eov[0:25, :, 0:1], 0.0)
            nc.gpsimd.memset(eov[0:25, :, 65:66], 0.0)
            # mask out-of-image rows: mult scores by rmt row-mask (free dim)
            nc.vector.tensor_tensor(
                eov[0:25, :, 1:65], eov[0:25, :, 1:65],
                rmt[0:25, :].unsqueeze(2).broadcast_to([25, RM, W]),
                op=ALU.mult)

            # Z = sum_k E -> Zr = rm / max(Z, 1)
            ones25 = per.tile([25, 1], FP16, tag="ones25")
            nc.gpsimd.memset(ones25[:], 1.0)
            zrv = zrow[:].rearrange("p (r w) -> p r w", r=RM)
            for g in range(3):
                c0 = g * 6 * EOW
                pz = psA.tile([1, 6 * EOW], F32, tag="cv", name=f"pz_{g}")
                nc.tensor.matmul(pz[:], ones25[:],
                                 EO[0:25, c0:c0 + 6 * EOW],
                                 start=True, stop=True)
                nc.vector.tensor_scalar_max(zrow[:, c0:c0 + 6 * EOW],
                                            pz[:], 1.0)
                nc.vector.reciprocal(zrow[:, c0:c0 + 6 * EOW],
                                     zrow[:, c0:c0 + 6 * EOW])
                nc.vector.tensor_tensor(
                    zrv[:, g * 6:g * 6 + 6, 1:65],
                    zrv[:, g * 6:g * 6 + 6, 1:65],
                    rmt[0:1, g * 6:g * 6 + 6].unsqueeze(2)
                    .broadcast_to([1, 6, W]),
                    op=ALU.mult)
                nc.gpsimd.dma_start(EO[25:26, c0:c0 + 6 * EOW],
                                    zrow[:, c0:c0 + 6 * EOW])

            # ---------------- transposes -> w-major ----------------
            # mTW[dxi][(t,w), (r, s14, q, p)], zW, offW
            ofv = offW[:].rearrange("p (r c) -> p r c", r=RM)
            for g in range(3):          # r batches of 6
                r0 = g * 6
                for dxi in range(3):
                    ptd = psT.tile([W, 6 * NCH], FP16, tag="ptd",
                                   name=f"ptd{dxi}_{g}", bufs=2)
                    for j in range(6):
                        r = r0 + j
                        nc.tensor.transpose(
                            ptd[:, j * NCH:(j + 1) * NCH],
                            EO[:, r * EOW + dxi:r * EOW + dxi + W],
                            idt[:])
                    pv = ptd[:].rearrange("w (j c) -> w j c", j=6)
                    mv4 = mTW[dxi][:].rearrange("p (r f s) -> p r f s",
                                                r=RM, f=4)
                    zv4 = zW[dxi][:].rearrange("p (r f) -> p r f", r=RM)
                    for t in range(2):
                        for jj in range(3):
                            if jj < 2:
                                c0 = 2 * jj + t
                                ssl = mv4[t * 64:(t + 1) * 64, r0:r0 + 6, :,
                                          jj * 5:jj * 5 + 5]
                                src = pv[:, :, c0:c0 + 21:5]  # 5 ki str 5
                            elif t == 0:
                                ssl = mv4[0:64, r0:r0 + 6, :, 10:13]
                                src = pv[:, :, 4:25:10]       # ki 0,2,4
                            else:
                                ssl = mv4[64:128, r0:r0 + 6, :, 10:12]
                                src = pv[:, :, 9:20:10]       # ki 1,3
                            nsl = ssl.shape[3]
                            if (t + jj) % 2 == 0:
                                nc.scalar.activation(
                                    ssl,
                                    src.unsqueeze(2)
                                    .broadcast_to([W, 6, 4, nsl]),
                                    ACTF.Copy)
                            else:
                                nc.vector.tensor_copy(
                                    ssl,
                                    src.unsqueeze(2)
                                    .broadcast_to([W, 6, 4, nsl]))
                        nc.scalar.activation(
                            zv4[t * 64:(t + 1) * 64, r0:r0 + 6, :],
                            pv[:, :, 25:26].broadcast_to([W, 6, 4]),
                            ACTF.Copy)
                        if dxi == 1:
                            nc.scalar.activation(
                                ofv[t * 64:(t + 1) * 64, r0:r0 + 6, :],
                                pv[:, :, 32:40], ACTF.Copy)
            for dxi in range(3):
                mv4 = mTW[dxi][:].rearrange("p (r f s) -> p r f s",
                                            r=RM, f=4)
                nc.gpsimd.memset(mv4[64:128, :, :, 12:13], 0.0)


            # ---------------- WGT (w-major) ----------------
            oyv = oyc[:].rearrange("p (h q e) -> p h q e", h=HB, q=2)
            oxv = oxc[:].rearrange("p (h q e) -> p h q e", h=HB, q=2)
            # oy chan = q*4+2+p, ox chan = q*4+0+p at mask row h+1
            oy_src = ofv[:, 1:1 + HB, :].rearrange("p h (q c) -> p h q c", q=2)
            nc.vector.tensor_tensor(
                oyv[:], oy_src[:, :, :, 2:4],
                ylo[:].rearrange("p (h q e) -> p h q e", h=HB, q=2),
                op=ALU.max)
            nc.vector.tensor_tensor(
                oyv[:], oyv[:],
                yhi[:].rearrange("p (h q e) -> p h q e", h=HB, q=2),
                op=ALU.min)
            nc.vector.tensor_scalar(oxv[:], oy_src[:, :, :, 0:2],
                                    xlo[:, 0:1], xhi[:, 0:1],
                                    op0=ALU.max, op1=ALU.min)
            # wy[dy, h, q, p] = relu(1 - |oyc - (dy-1)|)
            for (wt, oc) in ((wy, oyc), (wx, oxc)):
                wv = wt[:].rearrange("p (d f) -> p d f", d=3)
                tv = t192[:].rearrange("p (d f) -> p d f", d=3)
                for d in range(3):
                    nc.vector.tensor_scalar(tv[:, d, :], oc[:],
                                            float(1 - d), None, op0=ALU.add)
                nc.vector.tensor_scalar(wv[:], tv[:], -1.0, None,
                                        op0=ALU.mult)
                nc.vector.tensor_tensor(wv[:], wv[:], tv[:], op=ALU.max)
                nc.vector.tensor_scalar(wv[:], wv[:], -1.0, 1.0,
                                        op0=ALU.mult, op1=ALU.add)
                nc.vector.tensor_scalar(wv[:], wv[:], 0.0, None, op0=ALU.max)
            # wpr[dy,dx,(h,q,p)] = wy*wx*zr[h+dy, dx]
            nc.vector.tensor_tensor(
                wpr[:].rearrange("p (a b f) -> p a b f", a=3, b=3),
                wy[:].rearrange("p (a f) -> p a f", a=3).unsqueeze(2)
                .broadcast_to([128, 3, 3, HB * 4]),
                wx[:].rearrange("p (b f) -> p b f", b=3).unsqueeze(1)
                .broadcast_to([128, 3, 3, HB * 4]),
                op=ALU.mult)
            wprv4 = wpr[:].rearrange("p (a b h f) -> p a b h f", a=3, b=3,
                                     h=HB)
            for dy in range(3):
                for dx in range(3):
                    zs = zW[dx][:].rearrange("p (r f) -> p r f", r=RM)
                    nc.vector.tensor_tensor(
                        wprv4[:, dy, dx], wprv4[:, dy, dx],
                        zs[:, dy:dy + HB, :], op=ALU.mult)

            # ---------------- MU (muw) ----------------
            # muw layout (h, f4=(q,p), s13); mTW (r, f4, s14)
            muv = muw[:].rearrange("p (h f s) -> p h f s", h=HB, f=4)
            mutv = mut[:].rearrange("p (h f s) -> p h f s", h=HB, f=4)
            for hc in range(4):
                hl = hc * 4
                first = True
                for dy in range(3):
                    for dx in range(3):
                        msrc = mTW[dx][:].rearrange(
                            "p (r f s) -> p r f s", r=RM, f=4)[
                            :, dy + hl:dy + hl + 4, :, 0:NS]
                        wsl = wprv4[:, dy, dx, hl:hl + 4, :].unsqueeze(3) \
                            .broadcast_to([128, 4, 4, NS])
                        dst = muv if first else mutv
                        nc.vector.tensor_tensor(dst[:, hl:hl + 4], wsl,
                                                msrc, op=ALU.mult)
                        if not first:
                            nc.vector.tensor_tensor(
                                muv[:, hl:hl + 4], muv[:, hl:hl + 4],
                                mutv[:, hl:hl + 4], op=ALU.add)
                        first = False

            # ---------------- CARAFE ----------------
            idrv = idr[:].rearrange("p (q w g) -> p q w g", q=2, w=W)
            muq = muw[:].rearrange("p (h q g) -> p h q g", h=HB, q=2)
            for h in range(HB):
                st = strips[h % 3]
                stv = st[:].rearrange("p (q w g) -> p q w g", q=2, w=W)
                nc.vector.tensor_tensor(
                    stv[:],
                    muq[:, h, :, :].unsqueeze(2)
                    .broadcast_to([128, 2, W, 2 * NS]),
                    idrv[:], op=ALU.mult)
                for ch in range(2):
                    pc = psC.tile([128, 2 * 128], F32, tag="pc",
                                  name=f"pc{ch}_{h}", bufs=4)
                    x2v = None
                    stv2 = st[:].rearrange("p (q w e s) -> p q w e s",
                                           q=2, w=W, e=2)
                    for s in range(NS):
                        jj = s // 5 if s < 10 else 2
                        ki = s % 5 if s < 10 else 2 * (s - 10)
                        lhsT = X2[jj][:, (h + ki) * C + ch * 128:
                                      (h + ki) * C + ch * 128 + 128]
                        rhs = stv2[:, :, :, :, s].transpose([0, 3, 1, 2])
                        nc.tensor.matmul(pc[:], lhsT, rhs,
                                         start=(s == 0), stop=(s == NS - 1))
                    # psum free = (p, q, w) -> out[c, 2h+p, (w,q)]
                    oc = per.tile([128, 256], F32, tag=f"oc{ch}",
                                  name=f"oc{ch}_{h}", bufs=3)
                    nc.scalar.activation(
                        oc[:].rearrange("c (e w q) -> c e w q", e=2, q=2),
                        pc[:].rearrange("c (e q w) -> c e q w", e=2, q=2)
                        .transpose([0, 1, 3, 2]),
                        ACTF.Copy)
                    ov = out_sl.rearrange("c r w -> c (r w)")
                    (nc.sync if (h + ch) % 2 == 0 else nc.scalar).dma_start(
                        ov[ch * 128:(ch + 1) * 128,
                           2 * h * WOUT:(2 * h + 2) * WOUT],
                        oc[:])

            if debug:
                nc.sync.dma_start(dbg["d_comp"], comp[:])
                nc.sync.dma_start(dbg["d_EO"], EO[:])
                nc.sync.dma_start(dbg["d_mTW1"], mTW[1][:])
                nc.sync.dma_start(dbg["d_zW1"], zW[1][:])
                nc.sync.dma_start(dbg["d_offW"], offW[:])
                nc.sync.dma_start(dbg["d_wpr"], wpr[:])
                nc.sync.dma_start(dbg["d_muw"], muw[:])
                nc.sync.dma_start(dbg["d_strip0"], strips[0][:])
                nc.sync.dma_start(dbg["d_X2_0"], X2[0][:])

    nc.compile()
    return nc


def _consts(n, qh):
    h0 = qh * HB
    wv = np.arange(W)
    # ylo/yhi per (h,q,p): -(h0+h), 63-(h0+h)
    hvals = (h0 + np.arange(HB, dtype=np.float32))
    ylo = np.broadcast_to(np.repeat(-hvals, 4)[None, :], (128, HB * 4))
    yhi = np.broadcast_to(np.repeat(63.0 - hvals, 4)[None, :], (128, HB * 4))
    wq = np.tile(wv, 2).astype(np.float32)
    xlo = (-wq)[:, None].copy()
    xhi = (63.0 - wq)[:, None].copy()
    # IDrep2 [(t,w), (q, w', s, p)] = (w' == w%64) & valid(w', kj(t,s))
    idrep = np.zeros((128, 2, W, 2, NS), np.float32)
    for t in range(2):
        for s in range(NS):
            if s < 10:
                kj = 2 * (s // 5) + t
            else:
                if t == 1 and s == 12:
                    continue
                kj = 4
            for w in range(W):
                if 0 <= w + kj - 2 < W:
                    idrep[t * 64 + w, :, w, :, s] = 1.0
    # rmEO: mask row r (global h0-1+r) in image
    rm = np.array([[1.0 if 0 <= h0 - 1 + r < H else 0.0]
                   for r in range(RM)], np.float32).reshape(1, RM)
    rmc = np.zeros((CC, RX), np.float32)
    for r in range(RX):
        rmc[:, r] = 1.0 if 0 <= h0 - 2 + r < H else 0.0
    rm34 = np.broadcast_to(rm, (34, RM)).copy()
    return dict(ylo_h=ylo.astype(np.float16).copy(),
                yhi_h=yhi.astype(np.float16).copy(),
                xlo_h=xlo, xhi_h=xhi,
                idrep=idrep.reshape(128, -1).astype(np.float16),
                rmEO=rm34, rmc_h=rmc)


def _prep_in_maps(x, w_comp, b_comp, w_off, b_off, w_ker, b_ker):
    x = np.asarray(x, np.float32)
    w1l = np.asarray(w_comp, np.float32).reshape(CC, C).T.astype(
        ml_dtypes.bfloat16)
    perm = [xy * 4 + p * 2 + q for q in range(2) for xy in range(2)
            for p in range(2)]
    w2 = np.zeros((NCH, CC, 3, 3), np.float32)
    b2 = np.zeros((NCH,), np.float32)
    w2[0:25] = np.asarray(w_ker, np.float32)
    b2[0:25] = np.asarray(b_ker, np.float32)
    w2[32:40] = np.asarray(w_off, np.float32)[perm]
    b2[32:40] = np.asarray(b_off, np.float32)[perm]
    w2l = np.ascontiguousarray(
        w2.transpose(1, 2, 3, 0).reshape(CC, 9 * NCH)).astype(
        ml_dtypes.bfloat16)
    idt34 = np.eye(NCH, dtype=np.float16)

    in_maps = []
    for core in range(8):
        n, qh = core // QH, core % QH
        h0 = qh * HB
        lo, hi = h0 - 2, h0 + HB + 2
        slo, shi = max(lo, 0), min(hi, H)
        xs = np.zeros((C, RX, W), np.float32)
        xs[:, slo - lo:shi - lo] = x[n, :, slo:shi]
        xtb = np.ascontiguousarray(xs.transpose(2, 1, 0)).reshape(
            W, RX * C).astype(np.float16)
        im = dict(xs_h=xs.reshape(C, RX * W).astype(ml_dtypes.bfloat16),
                  xtb_h=xtb, w1l=w1l,
                  b1c=np.asarray(b_comp, np.float32)[:, None].copy(),
                  w2l=w2l, b2c=b2[:, None].copy(), idt34=idt34,
                  **_consts(n, qh))
        in_maps.append(im)
    return in_maps


def kernel(x, w_comp, b_comp, w_off, b_off, w_ker, b_ker):
    if "nc" not in _cache:
        _cache["nc"] = _build(debug=_cache.get("debug", False))
    nc = _cache["nc"]
    in_maps = _prep_in_maps(x, w_comp, b_comp, w_off, b_off, w_ker, b_ker)
    res = run_bass_kernel_spmd(nc, in_maps, core_ids=list(range(8)))
    _cache["last_res"] = res
    out = np.zeros((N, C, HOUT, WOUT), np.float32)
    for core in range(8):
        n, qh = core // QH, core % QH
        out[n, :, 2 * qh * HB:2 * (qh + 1) * HB] = res.results[core]["out_sl"]
    return out
